# revision 1
# baseline (speedup 1.0000x reference)
"""LlamaMoE (8 experts, top-2) on 8 Trainium2 cores.

Sharding: expert-parallel. Core e holds expert e's full weights and computes
its SwiGLU densely over all T=2048 tokens (output scaled per-token by the
router combine weight, 0 for tokens not routed to e), plus a 1/8
tensor-parallel slice of the always-on base MLP. The router (softmax + top-2
+ renorm) is computed on every core in exact fp32. Per-core partial outputs
are summed and token-sharded with on-device ReduceScatters (one per token
half, so the first overlaps second-half compute); the host only concatenates
slices.

Matmuls run in float32r (1 cycle/row) except the tiny router gate matmul
which uses exact float32 so top-2 selection matches the reference.

Loop structure: tokens split in 2 halves of 1024. Per half: router coefs,
then one sweep over all 25 gate/up weight-pair tiles (22 expert + 3 padded
base-TP), SwiGLU into fp32r `a` tiles, down-projection in ki-groups of <=4
accumulated in PSUM, scaled (expert groups only) and added into a resident
fp32 `osum` [128, 8x1024] accumulator, then DMA to the collective buffer.
"""

import numpy as np

import concourse.bass as bass
import concourse.mybir as mybir
import concourse.tile as tile
from concourse import bacc
from concourse.bass_utils import run_bass_kernel_spmd

N_CORES = 8
H = 1024          # hidden
I = 2816          # expert/base intermediate
E = 8             # experts
T = 2048          # tokens (B*S = 2*1024)
P = 128
KH = H // P       # 8 h-tiles
MI = I // P       # 22 intermediate pair-tiles per expert
ISL = I // N_CORES        # 352 base TP slice
ISLP = 384                # padded to 3*128
KB = ISLP // P            # 3
NM = MI + KB              # 25 pair tiles total per half sweep
TH = T // 2               # 1024 tokens per half
NTT = TH // P             # 8 token sub-tiles per half
NSUB = 2                  # 512-wide matmul sub-chunks per half
SUB = TH // NSUB          # 512
HC = 512                  # output h chunk for down matmul
NHC = H // HC             # 2

F32 = mybir.dt.float32
F32R = mybir.dt.float32r
AF = mybir.ActivationFunctionType
OP = mybir.AluOpType

# ki groups for down-projection accumulation: expert tiles then base tiles
GROUPS = [list(range(0, 4)), list(range(4, 8)), list(range(8, 12)),
          list(range(12, 16)), list(range(16, 20)), list(range(20, 22)),
          list(range(22, 25))]  # last group = base (unscaled)


def _build(reps=1, fresh_w=True, do_rs=True, timing_mode=False):
    nc = bacc.Bacc("TRN2", target_bir_lowering=False)

    gw_pk = nc.dram_tensor("gw_pk", [P, KH * E], F32, kind="ExternalInput")
    onehot = nc.dram_tensor("onehot", [P, E], F32, kind="ExternalInput")
    if timing_mode:
        # weights/activations as internal DRAM: nothing to stage per call, so
        # wall-clock slope over `reps` resolves true device time
        xh_pk = nc.dram_tensor("xh_pk_t", [2, P, KH * TH], F32R)
        wgu_pk = nc.dram_tensor("wgu_pk_t", [MI, P, 2 * KH * P], F32R)
        wd_pk = nc.dram_tensor("wd_pk_t", [P, MI * H], F32R)
        bgu_pk = nc.dram_tensor("bgu_pk_t", [KB, P, 2 * KH * P], F32R)
        bwd_pk = nc.dram_tensor("bwd_pk_t", [P, KB * H], F32R)
    else:
        xh_pk = nc.dram_tensor("xh_pk", [2, P, KH * TH], F32R, kind="ExternalInput")
        wgu_pk = nc.dram_tensor("wgu_pk", [MI, P, 2 * KH * P], F32R, kind="ExternalInput")
        wd_pk = nc.dram_tensor("wd_pk", [P, MI * H], F32R, kind="ExternalInput")
        bgu_pk = nc.dram_tensor("bgu_pk", [KB, P, 2 * KH * P], F32R, kind="ExternalInput")
        bwd_pk = nc.dram_tensor("bwd_pk", [P, KB * H], F32R, kind="ExternalInput")
    out_sl = nc.dram_tensor("out_sl", [T // N_CORES, H], F32, kind="ExternalOutput")

    with tile.TileContext(nc) as tc:
        with (
            tc.tile_pool(name="const", bufs=1) as cpool,
            tc.tile_pool(name="xp", bufs=1) as xpool,
            tc.tile_pool(name="os", bufs=1) as ospool,
            tc.tile_pool(name="wg", bufs=3) as wgpool,
            tc.tile_pool(name="wdp", bufs=6) as wdpool,
            tc.tile_pool(name="ap", bufs=8) as apool,
            tc.tile_pool(name="rt", bufs=2) as rtpool,
            tc.tile_pool(name="sgp", bufs=3) as sgpool,
            tc.tile_pool(name="ob", bufs=4) as opool,
            tc.tile_pool(name="ps_gu", bufs=2, space="PSUM") as ps_gu,
            tc.tile_pool(name="ps_dn", bufs=2, space="PSUM") as ps_dn,
            tc.tile_pool(name="ps_rt", bufs=2, space="PSUM") as ps_rt,
            tc.tile_pool(name="dram", bufs=1, space="DRAM") as dpool,
        ):
            # resident constants
            gw_sb = cpool.tile([P, KH * E], F32, tag="gw")
            nc.sync.dma_start(gw_sb[:], gw_pk[:])
            oh_sb = cpool.tile([P, E], F32, tag="oh")
            nc.sync.dma_start(oh_sb[:], onehot[:])
            bwd_sb = cpool.tile([P, KB * H], F32R, tag="bwd")
            nc.sync.dma_start(bwd_sb[:], bwd_pk[:])
            coef = cpool.tile([P, 2 * NTT], F32, tag="coef")

            cc_in = [dpool.tile([TH, H], F32, tag=f"ccin{h}", name=f"ccin{h}") for h in range(2)]
            cc_out = [dpool.tile([TH // N_CORES, H], F32, tag=f"ccout{h}", name=f"ccout{h}") for h in range(2)]

            for rep in range(reps):
              for half in range(2):
                xh = xpool.tile([P, KH * TH], F32R, tag="xh")
                nc.sync.dma_start(xh[:], xh_pk[half])

                # ---- router: coef[t] for this core's expert, per token sub-tile
                for tt in range(NTT):
                    lg_ps = ps_rt.tile([P, E], F32, tag="lg")
                    for k in range(KH):
                        nc.tensor.matmul(
                            out=lg_ps[:],
                            lhsT=xh[:, k * TH + tt * P : k * TH + (tt + 1) * P].bitcast(F32),
                            rhs=gw_sb[:, k * E : (k + 1) * E],
                            start=(k == 0),
                            stop=(k == KH - 1),
                        )
                    lg = rtpool.tile([P, E], F32, tag="lg_sb")
                    nc.vector.tensor_copy(lg[:], lg_ps[:])
                    mx = rtpool.tile([P, E], F32, tag="mx")
                    nc.vector.max(out=mx[:], in_=lg[:])
                    sc = rtpool.tile([P, 8], F32, tag="sc")
                    m1 = mx[:, 0:1]
                    m2 = mx[:, 1:2]
                    # l_sel = <logits, onehot>
                    nc.vector.tensor_tensor(out=sc[:, 0:E], in0=lg[:], in1=oh_sb[:], op=OP.mult)
                    lsel = rtpool.tile([P, 1], F32, tag="lsel")
                    nc.vector.reduce_sum(out=lsel[:], in_=sc[:, 0:E], axis=mybir.AxisListType.X)
                    # w1 = sigmoid(m1-m2), w2 = 1-w1
                    nc.vector.tensor_sub(out=sc[:, 0:1], in0=m1, in1=m2)
                    nc.scalar.activation(out=sc[:, 1:2], in_=sc[:, 0:1], func=AF.Sigmoid)
                    nc.vector.tensor_scalar(
                        out=sc[:, 2:3], in0=sc[:, 1:2], scalar1=-1.0, scalar2=1.0,
                        op0=OP.mult, op1=OP.add,
                    )
                    # coef = (lsel==m1)*w1 + (lsel==m2)*w2
                    nc.vector.tensor_tensor(out=sc[:, 3:4], in0=lsel[:], in1=m1, op=OP.is_equal)
                    nc.vector.tensor_tensor(out=sc[:, 4:5], in0=lsel[:], in1=m2, op=OP.is_equal)
                    nc.vector.tensor_tensor(out=sc[:, 5:6], in0=sc[:, 3:4], in1=sc[:, 1:2], op=OP.mult)
                    nc.vector.tensor_tensor(out=sc[:, 6:7], in0=sc[:, 4:5], in1=sc[:, 2:3], op=OP.mult)
                    nc.vector.tensor_add(
                        out=coef[:, half * NTT + tt : half * NTT + tt + 1],
                        in0=sc[:, 5:6], in1=sc[:, 6:7],
                    )

                osum = ospool.tile([P, NTT * H], F32, tag="osum")

                for gi, grp in enumerate(GROUPS):
                    is_base = grp[0] >= MI
                    # ---- gate/up + SwiGLU for this group's pair tiles
                    a_tiles = {}
                    for m in grp:
                        if fresh_w or (rep == 0 and half == 0 and gi == 0 and m == grp[0]):
                            wg = wgpool.tile([P, 2 * KH * P], F32R, tag="wg", name="wg")
                            if m < MI:
                                nc.sync.dma_start(wg[:], wgu_pk[m])
                            else:
                                nc.sync.dma_start(wg[:], bgu_pk[m - MI])
                            if not fresh_w:
                                wg_fixed = wg
                        if not fresh_w:
                            wg = wg_fixed
                        a_m = apool.tile([P, TH], F32R, tag="a")
                        a_tiles[m] = a_m
                        # k-outer so each 128-col weight slab is reused for both
                        # 512-token sub-chunks (halves LDWEIGHTS traffic)
                        g_ps = [ps_gu.tile([P, SUB], F32, tag=f"g{s}", name=f"g{s}", bufs=1)
                                for s in range(NSUB)]
                        u_ps = [ps_gu.tile([P, SUB], F32, tag=f"u{s}", name=f"u{s}", bufs=1)
                                for s in range(NSUB)]
                        for k in range(KH):
                            for s in range(NSUB):
                                nc.tensor.matmul(
                                    out=g_ps[s][:],
                                    lhsT=wg[:, k * P : (k + 1) * P],
                                    rhs=xh[:, k * TH + s * SUB : k * TH + (s + 1) * SUB],
                                    start=(k == 0),
                                    stop=(k == KH - 1),
                                )
                        for k in range(KH):
                            for s in range(NSUB):
                                nc.tensor.matmul(
                                    out=u_ps[s][:],
                                    lhsT=wg[:, (KH + k) * P : (KH + k + 1) * P],
                                    rhs=xh[:, k * TH + s * SUB : k * TH + (s + 1) * SUB],
                                    start=(k == 0),
                                    stop=(k == KH - 1),
                                )
                        for s in range(NSUB):
                            sg = sgpool.tile([P, SUB], F32, tag="sg")
                            nc.scalar.activation(out=sg[:], in_=g_ps[s][:], func=AF.Silu)
                            nc.vector.tensor_tensor(
                                out=a_m[:, s * SUB : (s + 1) * SUB],
                                in0=sg[:], in1=u_ps[s][:], op=OP.mult,
                            )

                    # ---- down projection for this group, accumulate into osum
                    wd_of = {}
                    for ki in grp:
                        if ki < MI:
                            if fresh_w or (rep == 0 and half == 0 and ki == 0):
                                wdk = wdpool.tile([P, H], F32R, tag="wd", name="wd")
                                nc.sync.dma_start(wdk[:], wd_pk[:, ki * H : (ki + 1) * H])
                                if not fresh_w:
                                    wd_fixed = wdk
                            if not fresh_w:
                                wdk = wd_fixed
                            wd_of[ki] = wdk
                    for tt in range(NTT):
                        for hc in range(NHC):
                            e_ps = ps_dn.tile([P, HC], F32, tag="e")
                            for j, ki in enumerate(grp):
                                if ki < MI:
                                    rhs = wd_of[ki][:, hc * HC : (hc + 1) * HC]
                                else:
                                    kb = ki - MI
                                    rhs = bwd_sb[:, kb * H + hc * HC : kb * H + (hc + 1) * HC]
                                nc.tensor.matmul(
                                    out=e_ps[:],
                                    lhsT=a_tiles[ki][:, tt * P : (tt + 1) * P],
                                    rhs=rhs,
                                    start=(j == 0),
                                    stop=(j == len(grp) - 1),
                                )
                            osl = osum[:, tt * H + hc * HC : tt * H + (hc + 1) * HC]
                            if is_base:
                                nc.vector.tensor_add(out=osl, in0=osl, in1=e_ps[:])
                            else:
                                ob = opool.tile([P, HC], F32, tag="ob")
                                nc.vector.tensor_scalar(
                                    out=ob[:], in0=e_ps[:],
                                    scalar1=coef[:, half * NTT + tt : half * NTT + tt + 1],
                                    scalar2=None, op0=OP.mult,
                                )
                                if gi == 0:
                                    nc.vector.tensor_copy(osl, ob[:])
                                else:
                                    nc.vector.tensor_add(out=osl, in0=osl, in1=ob[:])

                # ---- store half partial + reduce-scatter
                for tt in range(NTT):
                    nc.sync.dma_start(
                        cc_in[half][tt * P : (tt + 1) * P, :],
                        osum[:, tt * H : (tt + 1) * H],
                    )
                if do_rs:
                    nc.gpsimd.collective_compute(
                        "ReduceScatter",
                        OP.add,
                        replica_groups=[list(range(N_CORES))],
                        ins=[cc_in[half][:].opt()],
                        outs=[cc_out[half][:].opt()],
                    )
                    nc.sync.dma_start(
                        out_sl[half * (TH // N_CORES) : (half + 1) * (TH // N_CORES), :],
                        cc_out[half][:],
                    )
                else:
                    nc.sync.dma_start(
                        out_sl[half * (TH // N_CORES) : (half + 1) * (TH // N_CORES), :],
                        osum[:, :H],
                    )

    nc.compile()
    return nc


_CACHE = {}


def _pack_inputs(x, gate_w, base_wgu, base_wd, exp_wgu, exp_wd):
    xt = np.ascontiguousarray(np.asarray(x, np.float32).reshape(T, H))
    # [half, p, k*c] where xh[h, p, k*TH + c] = x[h*TH + c, k*P + p]
    xh_pk = np.ascontiguousarray(
        xt.reshape(2, TH, KH, P).transpose(0, 3, 2, 1).reshape(2, P, KH * TH)
    )
    gw_pk = np.ascontiguousarray(
        np.asarray(gate_w, np.float32).reshape(KH, P, E).transpose(1, 0, 2).reshape(P, KH * E)
    )

    def pack_gu(w):  # w [H, 2*mi*P] gate|up halves -> [mi, P, 2*KH*P]
        half = w.shape[1] // 2
        mi = half // P
        g = w[:, :half].reshape(KH, P, mi, P)
        u = w[:, half:].reshape(KH, P, mi, P)
        pk = np.stack([g, u], axis=0)  # [gu, k, p, m, c]
        return np.ascontiguousarray(pk.transpose(3, 2, 0, 1, 4).reshape(mi, P, 2 * KH * P))

    def pack_wd(w):  # w [ki*P, H] -> [P, ki*H] inner [hc, c]
        ki = w.shape[0] // P
        return np.ascontiguousarray(
            w.reshape(ki, P, NHC, HC).transpose(1, 0, 2, 3).reshape(P, ki * H)
        )

    per_core = []
    for e in range(N_CORES):
        sl = slice(e * ISL, (e + 1) * ISL)
        bgu = np.zeros((H, 2 * ISLP), np.float32)
        bgu[:, :ISL] = np.asarray(base_wgu, np.float32)[:, :I][:, sl]
        bgu[:, ISLP : ISLP + ISL] = np.asarray(base_wgu, np.float32)[:, I:][:, sl]
        bwd = np.zeros((ISLP, H), np.float32)
        bwd[:ISL] = np.asarray(base_wd, np.float32)[sl, :]
        onehot = np.zeros((P, E), np.float32)
        onehot[:, e] = 1.0
        per_core.append(
            {
                "xh_pk": xh_pk,
                "gw_pk": gw_pk,
                "onehot": onehot,
                "wgu_pk": pack_gu(np.asarray(exp_wgu[e], np.float32)),
                "wd_pk": pack_wd(np.asarray(exp_wd[e], np.float32)),
                "bgu_pk": pack_gu(bgu),
                "bwd_pk": pack_wd(bwd),
            }
        )
    return per_core


def _get_nc():
    if "nc" not in _CACHE:
        _CACHE["nc"] = _build()
    return _CACHE["nc"]


def _unshard(results, shape, dtype):
    y = np.empty((T, H), np.float32)
    q = TH // N_CORES  # 128
    for c in range(N_CORES):
        o = results[c]["out_sl"]
        y[c * q : (c + 1) * q] = o[:q]
        y[TH + c * q : TH + (c + 1) * q] = o[q:]
    return y.reshape(shape).astype(dtype)


def kernel(x, gate_w, base_wgu, base_wd, exp_wgu, exp_wd):
    nc = _get_nc()
    in_maps = _pack_inputs(x, gate_w, base_wgu, base_wd, exp_wgu, exp_wd)
    res = run_bass_kernel_spmd(nc, in_maps, core_ids=list(range(N_CORES)))
    return _unshard(res.results, x.shape, x.dtype)



# revision 5
# speedup vs baseline: 118.2729x; 118.2729x over previous
"""LlamaMoE (8 experts, top-2) on 8 Trainium2 cores — routed expert-parallel.

Sharding: the host computes the router (softmax + top-2 + renorm, float64 —
selection-identical to the reference's fp32 jax top_k for any non-degenerate
gap) and gathers each expert's assigned tokens into a padded capacity-C
buffer. Core e computes ONLY its expert's SwiGLU over its <=C gathered
tokens (scaled on-device by the per-token combine weight), plus a 1/8
token-shard of the always-on base MLP (256 tokens). No collectives: the
host scatter-adds the per-expert outputs and concatenates base slices.
This cuts expert matmul work 4x vs dense (top-2 of 8).

All matmuls in bf16 (fp32 PSUM accumulation): halves HBM weight traffic and
enables fast-weight-load; rel err budget (2e-2) leaves ~4x margin.

Per-core PE work: 22 gate/up pair-tiles x (8k x 1152 rows) + down
10x(22x512) for the expert pass, plus the same structure at 256 tokens for
the base pass — ~450k PE rows ~ 188us at 2.4GHz, vs 1216k dense-fp32r rows
in the previous dense kernel.

PSUM: gate|up packed in one [128, 2C] fp32 tile (3 banks, double-buffered =
6) with matmul sub-chunks split at bank boundaries; down uses [128,512] x2.
"""

import numpy as np
import ml_dtypes

import concourse.bass as bass
import concourse.mybir as mybir
import concourse.tile as tile
from concourse import bacc
from concourse.bass_utils import run_bass_kernel_spmd

N_CORES = 8
H = 1024          # hidden
I = 2816          # intermediate
E = 8             # experts
T = 2048          # tokens (B*S)
P = 128
KH = H // P       # 8 h-tiles
MI = I // P       # 22 intermediate pair-tiles
CB = T // N_CORES // 1  # 256 base tokens per core (token-sharded base)
NTB = CB // P     # 2
HC = 512          # down-proj output column chunk (1 PSUM bank)
DEF_C = 576       # default expert token capacity (max load 551 @ seed 0)

F32 = mybir.dt.float32
BF16 = mybir.dt.bfloat16
AF = mybir.ActivationFunctionType
OP = mybir.AluOpType
BNK = 512         # fp32 elements per PSUM bank


def _segs(lo, hi):
    """Split [lo, hi) at PSUM bank boundaries (multiples of BNK fp32)."""
    pts = [lo] + [b for b in range((lo // BNK + 1) * BNK, hi, BNK)] + [hi]
    return list(zip(pts[:-1], pts[1:]))


def _build(C=DEF_C, reps=1, timing_mode=False):
    nc = bacc.Bacc("TRN2", target_bir_lowering=False)
    NTT = (C + P - 1) // P

    coef_pk = nc.dram_tensor("coef_pk", [P, NTT], F32, kind="ExternalInput")
    if timing_mode:
        # weights/activations as internal DRAM: nothing staged per call, so
        # wall-clock slope over `reps` resolves true device time
        xg_pk = nc.dram_tensor("xg_pk_t", [P, KH * C], BF16)
        xb_pk = nc.dram_tensor("xb_pk_t", [P, KH * CB], BF16)
        wgu_pk = nc.dram_tensor("wgu_pk_t", [MI, P, 2 * KH * P], BF16)
        wd_pk = nc.dram_tensor("wd_pk_t", [P, MI * H], BF16)
        bgu_pk = nc.dram_tensor("bgu_pk_t", [MI, P, 2 * KH * P], BF16)
        bwd_pk = nc.dram_tensor("bwd_pk_t", [P, MI * H], BF16)
    else:
        xg_pk = nc.dram_tensor("xg_pk", [P, KH * C], BF16, kind="ExternalInput")
        xb_pk = nc.dram_tensor("xb_pk", [P, KH * CB], BF16, kind="ExternalInput")
        wgu_pk = nc.dram_tensor("wgu_pk", [MI, P, 2 * KH * P], BF16, kind="ExternalInput")
        wd_pk = nc.dram_tensor("wd_pk", [P, MI * H], BF16, kind="ExternalInput")
        bgu_pk = nc.dram_tensor("bgu_pk", [MI, P, 2 * KH * P], BF16, kind="ExternalInput")
        bwd_pk = nc.dram_tensor("bwd_pk", [P, MI * H], BF16, kind="ExternalInput")
    out_y = nc.dram_tensor("out_y", [C, H], F32, kind="ExternalOutput")
    out_b = nc.dram_tensor("out_b", [CB, H], F32, kind="ExternalOutput")

    with tile.TileContext(nc) as tc:
        with (
            tc.tile_pool(name="const", bufs=1) as cpool,
            tc.tile_pool(name="xp", bufs=1) as xpool,
            tc.tile_pool(name="wg", bufs=4) as wgpool,
            tc.tile_pool(name="wdp", bufs=1) as wdpool,
            tc.tile_pool(name="ap", bufs=1) as apool,
            tc.tile_pool(name="sgp", bufs=4) as sgpool,
            tc.tile_pool(name="ob", bufs=2) as opool,
            tc.tile_pool(name="ps_gu", bufs=3, space="PSUM") as ps_gu,
            tc.tile_pool(name="ps_dn", bufs=2, space="PSUM") as ps_dn,
        ):
            coef = cpool.tile([P, NTT], F32, tag="coef")
            nc.sync.dma_start(coef[:], coef_pk[:])
            # statically-resident activation tiles (22 live at once per pass)
            a_sb = [apool.tile([P, C], BF16, tag=f"a{m}", name=f"a{m}")
                    for m in range(MI)]
            ab_sb = [apool.tile([P, CB], BF16, tag=f"ab{m}", name=f"ab{m}")
                     for m in range(MI)]

            def swiglu_gu(m, wsrc, xh, ntok, a_out):
                """gate/up pair-tile m: matmul + SwiGLU -> a_out [P, ntok] bf16.

                PSUM `start` clears the whole bank, so every concurrently-
                accumulating region gets its own bank: one [P, BNK] tile per
                512-token chunk for g and for u."""
                wg = wgpool.tile([P, 2 * KH * P], BF16, tag="wg", name="wg")
                nc.sync.dma_start(wg[:], wsrc[m])
                chunks = _segs(0, ntok)
                g_ps = [ps_gu.tile([P, BNK], F32, tag="g", name="g_ps")
                        for _ in chunks]
                u_ps = [ps_gu.tile([P, BNK], F32, tag="u", name="u_ps")
                        for _ in chunks]
                for k in range(KH):
                    for ci, (lo, hi) in enumerate(chunks):
                        nc.tensor.matmul(
                            out=g_ps[ci][:, : hi - lo],
                            lhsT=wg[:, k * P:(k + 1) * P],
                            rhs=xh[:, k * ntok + lo:k * ntok + hi],
                            start=(k == 0), stop=(k == KH - 1),
                        )
                    for ci, (lo, hi) in enumerate(chunks):
                        nc.tensor.matmul(
                            out=u_ps[ci][:, : hi - lo],
                            lhsT=wg[:, (KH + k) * P:(KH + k + 1) * P],
                            rhs=xh[:, k * ntok + lo:k * ntok + hi],
                            start=(k == 0), stop=(k == KH - 1),
                        )
                for ci, (lo, hi) in enumerate(chunks):
                    cw = hi - lo
                    sg = sgpool.tile([P, BNK], F32, tag="sg", name="sg")
                    nc.scalar.activation(out=sg[:, :cw], in_=g_ps[ci][:, :cw],
                                         func=AF.Silu)
                    nc.vector.tensor_tensor(
                        out=a_out[:, lo:hi], in0=sg[:, :cw],
                        in1=u_ps[ci][:, :cw], op=OP.mult,
                    )

            def down(a_tiles, wd_sb, ntt, last_w, out_dram, scaled):
                for tt in range(ntt):
                    w = last_w if tt == ntt - 1 else P
                    ob = opool.tile([P, H], F32, tag="ob", name="ob")
                    for hc in range(2):
                        dn = ps_dn.tile([P, HC], F32, tag="dn", name="dn")
                        for ki in range(MI):
                            nc.tensor.matmul(
                                out=dn[:w],
                                lhsT=a_tiles[ki][:, tt * P:tt * P + w],
                                rhs=wd_sb[:, ki * H + hc * HC:ki * H + (hc + 1) * HC],
                                start=(ki == 0), stop=(ki == MI - 1),
                            )
                        osl = ob[:w, hc * HC:(hc + 1) * HC]
                        if scaled:
                            nc.vector.tensor_scalar(
                                out=osl, in0=dn[:w],
                                scalar1=coef[:w, tt:tt + 1], scalar2=None,
                                op0=OP.mult,
                            )
                        else:
                            nc.vector.tensor_copy(osl, dn[:w])
                    nc.sync.dma_start(out_dram[tt * P:tt * P + w, :], ob[:w])

            for rep in range(reps):
                xg = xpool.tile([P, KH * C], BF16, tag="xg", name="xg")
                nc.sync.dma_start(xg[:], xg_pk[:])
                xb = xpool.tile([P, KH * CB], BF16, tag="xb", name="xb")
                nc.sync.dma_start(xb[:], xb_pk[:])
                wd_sb = wdpool.tile([P, MI * H], BF16, tag="wd", name="wd")
                bwd_sb = wdpool.tile([P, MI * H], BF16, tag="bwd", name="bwd")

                # ---- expert gate/up (wd chunks prefetched alongside)
                for m in range(MI):
                    nc.sync.dma_start(wd_sb[:, m * H:(m + 1) * H],
                                      wd_pk[:, m * H:(m + 1) * H])
                    swiglu_gu(m, wgu_pk, xg, C, a_sb[m])
                # ---- expert down (scaled by router coef)
                down(a_sb, wd_sb, (C + P - 1) // P, C - P * ((C - 1) // P),
                     out_y, scaled=True)
                # ---- base gate/up (bwd chunks prefetched alongside)
                for m in range(MI):
                    nc.sync.dma_start(bwd_sb[:, m * H:(m + 1) * H],
                                      bwd_pk[:, m * H:(m + 1) * H])
                    swiglu_gu(m, bgu_pk, xb, CB, ab_sb[m])
                # ---- base down (unscaled)
                down(ab_sb, bwd_sb, NTB, P, out_b, scaled=False)

    nc.compile()
    return nc


_CACHE = {}


def _get_nc(C=DEF_C):
    if C not in _CACHE:
        _CACHE[C] = _build(C=C)
    return _CACHE[C]


def _route(xt, gate_w):
    """Host router in float64: top-2 experts + renormalized weights.

    Selection matches the reference's fp32 jax top_k whenever the top2/top3
    prob gap exceeds fp32 rounding noise (~1e-6; min gap for this input is
    5e-5)."""
    logits = xt.astype(np.float64) @ np.asarray(gate_w, np.float64)
    m = logits.max(-1, keepdims=True)
    p = np.exp(logits - m)
    p /= p.sum(-1, keepdims=True)
    t1 = np.argmax(p, axis=-1)
    rows = np.arange(p.shape[0])
    p2 = p.copy()
    p2[rows, t1] = -1.0
    t2 = np.argmax(p2, axis=-1)
    w1 = p[rows, t1]
    w2 = p[rows, t2]
    s = w1 + w2
    return t1, t2, (w1 / s).astype(np.float32), (w2 / s).astype(np.float32)


def _pack_x(xrows, ntok):
    """[n<=ntok, H] f32 -> [P, KH*ntok] bf16 with xp[p, k*ntok+c] = x[c, k*P+p]."""
    buf = np.zeros((ntok, H), np.float32)
    buf[: xrows.shape[0]] = xrows
    return np.ascontiguousarray(
        buf.reshape(ntok, KH, P).transpose(2, 1, 0).reshape(P, KH * ntok)
    ).astype(ml_dtypes.bfloat16)


def _pack_gu(w):
    """[H, 2I] f32 -> [MI, P, 2*KH*P] bf16; slab (m, gu, k) at [m,:,(gu*KH+k)*P:]."""
    g = w[:, :I].reshape(KH, P, MI, P)
    u = w[:, I:].reshape(KH, P, MI, P)
    pk = np.stack([g, u], axis=0)  # [gu, k, p, m, c]
    return np.ascontiguousarray(
        pk.transpose(3, 2, 0, 1, 4).reshape(MI, P, 2 * KH * P)
    ).astype(ml_dtypes.bfloat16)


def _pack_wd(w):
    """[I, H] f32 -> [P, MI*H] bf16 with wd[p, ki*H+h] = w[ki*P+p, h]."""
    return np.ascontiguousarray(
        w.reshape(MI, P, H).transpose(1, 0, 2).reshape(P, MI * H)
    ).astype(ml_dtypes.bfloat16)


def _prepare(x, gate_w, base_wgu, base_wd, exp_wgu, exp_wd):
    xt = np.ascontiguousarray(np.asarray(x, np.float32).reshape(T, H))
    t1, t2, w1, w2 = _route(xt, gate_w)

    idxs, coefs = [], []
    for e in range(E):
        m1 = t1 == e
        m2 = t2 == e
        idx = np.where(m1 | m2)[0]
        cw = np.where(m1[idx], w1[idx], w2[idx]).astype(np.float32)
        idxs.append(idx)
        coefs.append(cw)
    C = max(DEF_C, max((len(i) + 63) // 64 * 64 for i in idxs))
    NTT = (C + P - 1) // P

    bgu_pk = _pack_gu(np.asarray(base_wgu, np.float32))
    bwd_pk = _pack_wd(np.asarray(base_wd, np.float32))
    ewgu = np.asarray(exp_wgu, np.float32)
    ewd = np.asarray(exp_wd, np.float32)

    per_core = []
    for e in range(E):
        cw = coefs[e]
        col = np.zeros(NTT * P, np.float32)
        col[: len(cw)] = cw
        coef_pk = col.reshape(NTT, P).T  # [p, tt] = cw[tt*P+p]
        per_core.append({
            "coef_pk": np.ascontiguousarray(coef_pk),
            "xg_pk": _pack_x(xt[idxs[e]], C),
            "xb_pk": _pack_x(xt[e * CB:(e + 1) * CB], CB),
            "wgu_pk": _pack_gu(ewgu[e]),
            "wd_pk": _pack_wd(ewd[e]),
            "bgu_pk": bgu_pk,
            "bwd_pk": bwd_pk,
        })
    return per_core, idxs, C


def kernel(x, gate_w, base_wgu, base_wd, exp_wgu, exp_wd):
    per_core, idxs, C = _prepare(x, gate_w, base_wgu, base_wd, exp_wgu, exp_wd)
    nc = _get_nc(C)
    res = run_bass_kernel_spmd(nc, per_core, core_ids=list(range(N_CORES)))
    y = np.empty((T, H), np.float32)
    for c in range(N_CORES):
        y[c * CB:(c + 1) * CB] = res.results[c]["out_b"]
    for e in range(E):
        y[idxs[e]] += res.results[e]["out_y"][: len(idxs[e])]
    return y.reshape(np.asarray(x).shape).astype(np.asarray(x).dtype)
